# revision 33
# baseline (speedup 1.0000x reference)
"""Trainium2 Bass kernel for nn_BatchDelayProcessor.

Computes, per batch row (B=64, T=441000, D=22050 delay, 20 blocks):
    delayed[t] = 0                          , t < D
    delayed[t] = x[t-D] + 0.3*delayed[t-D]  , t >= D
    out[t]     = 0.5*x[t] + 0.5*delayed[t]

With u_k = 0.5*x_k (folded into the host-side bf16 conversion -- an exact
exponent shift), the block recurrence unrolls to a dense lower-triangular
combination:
    out_m = u_m + sum_{j<m} 0.3^(m-1-j) * u_j  =  sum_j A[m,j] u_j
so the whole kernel is ONE 20x20 matrix applied per sample position --
perfect for the otherwise-idle PE array, with NO serial chain anywhere.
Six independent position-groups are folded into a 120x120 block-diagonal
stationary, so each matmul tile computes 6 groups x 20 blocks at once.

Pipeline (per core): loads -> PE (60 tiles of 490 cols) -> PSUM drain
(split DVE even tiles / ACT odd tiles, f32->bf16 downcast) -> stores.

Schedule notes (evidence from perfetto traces of 8 prior variants):
  - bf16 I/O halves HBM traffic to 7.06 MB each way (tol 2e-2, measured
    err 3.8e-3).
  - ALL DMA on the single sync HWDGE ring, loads (consumption order,
    telescoping sizes) ahead of stores (drain-gated): HWDGE descriptors
    execute at full ~26.5 GB/s/engine vs ~15 for SWDGE ones, one ring's
    strict FIFO avoids the cross-queue round-robin that starved head
    groups in two-queue variants, and loads-then-stores serializes HBM
    into pure-read then pure-write phases.
  - Each load/store group is its OWN dense DRAM parameter (120 x cols
    contiguous): with a strided (120, 29400) layout, per-descriptor
    reads jumped 58.8 KB apart and HBM read rate dropped to ~15
    GB/s/engine; dense per-DMA regions restore line rate.

Sharding: data-parallel over batch -- 8 rows per NeuronCore, no comms.
Layout: partition p = g*20 + k holds block k of position-group g; its
29400 columns are sample positions (flattened (row, d), split into 6
contiguous groups of 29400).
"""

from contextlib import ExitStack

import ml_dtypes
import numpy as np

import concourse.bass as bass
import concourse.mybir as mybir
from concourse.bass_utils import run_bass_kernel_spmd

B, T = 64, 441000
D, NBLK = 22050, 20
NCORES = 8
ROWS = B // NCORES          # 8 rows per core
NG = 6                      # position-groups folded into the stationary
P = NG * NBLK               # 120 partitions
PW = ROWS * D // NG         # 29400 positions per group-lane

TN = 490                    # moving-tile columns (1960 B psum = 1 bank)
NT = PW // TN               # 60 tiles
NPB = 8                     # psum bank ring

# telescoping load groups (units: tiles); small at BOTH ends: fast PE
# start, and only ~2us of PE work left after the last load byte lands
# (a 12-tile last group left ~4.4us of tail compute)
LOAD_TILES = [3, 3, 6, 12, 12, 12, 6, 3, 3]
# store groups (tiles); boundaries must be even (drain-parity gating);
# tiny last group shrinks the final write-receipt tail
STORE_TILES = [12, 12, 12, 12, 6, 4, 2]
FEEDBACK, MIX = 0.3, 0.5

BF16 = mybir.dt.bfloat16
F32 = mybir.dt.float32
BF16_NP = ml_dtypes.bfloat16


def _amat() -> np.ndarray:
    """Block-diag stationary lhsT[(g,k),(g',m)] = A[m,k] * (g==g')."""
    A = np.zeros((NBLK, NBLK), dtype=np.float64)
    for m in range(NBLK):
        A[m, m] = 1.0
        for j in range(m):
            A[m, j] = FEEDBACK ** (m - 1 - j)
    lhsT = np.zeros((P, P), dtype=np.float64)
    for g in range(NG):
        lhsT[g * NBLK : (g + 1) * NBLK, g * NBLK : (g + 1) * NBLK] = A.T
    return lhsT.astype(BF16_NP)


def build_nc() -> bass.Bass:
    assert sum(LOAD_TILES) == NT and sum(STORE_TILES) == NT
    load_hi = np.cumsum(LOAD_TILES).tolist()      # exclusive upper tile
    store_lo = [0] + np.cumsum(STORE_TILES).tolist()[:-1]

    nc = bass.Bass(trn_type="TRN2")
    a = nc.declare_dram_parameter("a", [P, P], BF16, isOutput=False)
    # one dense DRAM tensor per load group / store group
    xs = [
        nc.declare_dram_parameter(f"x{g}", [P, nt * TN], BF16, isOutput=False)
        for g, nt in enumerate(LOAD_TILES)
    ]
    ys = [
        nc.declare_dram_parameter(f"y{s}", [P, nt * TN], BF16, isOutput=True)
        for s, nt in enumerate(STORE_TILES)
    ]

    with ExitStack() as ctx:
        block = ctx.enter_context(nc.Block())
        abuf = ctx.enter_context(nc.sbuf_tensor("abuf", [P, P], BF16))
        xbuf = ctx.enter_context(nc.sbuf_tensor("xbuf", [P, PW], BF16))
        obuf = ctx.enter_context(nc.sbuf_tensor("obuf", [P, PW], BF16))
        pb = [
            ctx.enter_context(nc.psum_tensor(f"pb{j}", [P, TN], F32))
            for j in range(NPB)
        ]
        s_a = ctx.enter_context(nc.semaphore("s_a"))
        s_in = [
            ctx.enter_context(nc.semaphore(f"s_in{g}"))
            for g in range(len(LOAD_TILES))
        ]
        s_pe = ctx.enter_context(nc.semaphore("s_pe"))
        s_dve = ctx.enter_context(nc.semaphore("s_dve"))
        s_act = ctx.enter_context(nc.semaphore("s_act"))
        s_st = ctx.enter_context(nc.semaphore("s_st"))

        def tcols(buf, t0, t1):
            return buf[:, t0 * TN : t1 * TN]

        # Loads are split BY PARTITION HALVES across the two HWDGE
        # rings: both rings carry identical byte schedules in the same
        # group order, and the PE waits for both halves (sem >= 32), so
        # cross-queue arbitration skew cannot starve the next-needed
        # group (which is what sank every BY-GROUP two-queue split).
        # This probes whether the ~200 GB/s read rate is a per-queue
        # limit; stores queue on the sync ring behind its half-loads.
        PH = P // 2

        @block.sync
        def _(sync):
            sync.dma_start(out=abuf[:, :], in_=a[:, :]).then_inc(s_a, 16)
            t0 = 0
            for g, t1 in enumerate(load_hi):
                sync.dma_start(
                    out=xbuf[0:PH, t0 * TN : t1 * TN],
                    in_=xs[g][0:PH, :],
                ).then_inc(s_in[g], 16)
                t0 = t1
            for s, st0 in enumerate(store_lo):
                st1 = st0 + STORE_TILES[s]
                sync.wait_ge(s_dve, st1 // 2)
                sync.wait_ge(s_act, st1 // 2)
                sync.dma_start(
                    out=ys[s][:, :], in_=tcols(obuf, st0, st1)
                ).then_inc(s_st, 16)

        # PE: one self-loading matmul per 490-column tile against the
        # block-diagonal stationary; psum banks cycle mod 8.
        @block.tensor
        def _(tensor):
            tensor.wait_ge(s_a, 16)
            g = -1
            for t in range(NT):
                gt = next(i for i, hi in enumerate(load_hi) if t < hi)
                if gt > g:
                    tensor.wait_ge(s_in[gt], 32)  # both partition halves
                    g = gt
                if t >= NPB:
                    td = t - NPB  # bank WAR: tile td's drain read the bank
                    if td % 2 == 0:
                        tensor.wait_ge(s_dve, td // 2 + 1)
                    else:
                        tensor.wait_ge(s_act, td // 2 + 1)
                nc.tensor.matmul(
                    out=pb[t % NPB][:, :],
                    lhsT=abuf[:, :],
                    rhs=tcols(xbuf, t, t + 1),
                    start=True,
                    stop=True,
                ).then_inc(s_pe, 1)

        # PSUM drain, split across DVE (even tiles) and ACT (odd tiles);
        # both do the f32 -> bf16 downcast into obuf.
        @block.vector
        def _(vector):
            for t in range(0, NT, 2):
                vector.wait_ge(s_pe, t + 1)
                nc.vector.tensor_copy(
                    tcols(obuf, t, t + 1), pb[t % NPB][:, :]
                ).then_inc(s_dve, 1)

        @block.scalar
        def _(scalar):
            # upper partition halves of every load group, same order as
            # the sync ring's lower halves; all emitted up-front before
            # the drain loop needs this sequencer
            t0 = 0
            for g, t1 in enumerate(load_hi):
                scalar.dma_start(
                    out=xbuf[PH:P, t0 * TN : t1 * TN],
                    in_=xs[g][PH:P, :],
                ).then_inc(s_in[g], 16)
                t0 = t1
            for t in range(1, NT, 2):
                scalar.wait_ge(s_pe, t + 1)
                nc.scalar.copy(
                    tcols(obuf, t, t + 1), pb[t % NPB][:, :]
                ).then_inc(s_act, 1)

    return nc


_NC_CACHE = None


def _get_nc() -> bass.Bass:
    global _NC_CACHE
    if _NC_CACHE is None:
        _NC_CACHE = build_nc()
    return _NC_CACHE


_A_BF16 = _amat()
_LOAD_HI = np.cumsum(LOAD_TILES).tolist()
_STORE_HI = np.cumsum(STORE_TILES).tolist()


def _shard(x: np.ndarray) -> list[dict[str, np.ndarray]]:
    x = np.asarray(x, dtype=np.float32)
    assert x.shape == (B, T), x.shape
    maps = []
    for i in range(NCORES):
        u = (x[i * ROWS : (i + 1) * ROWS] * np.float32(MIX)).reshape(
            ROWS, NBLK, D
        )
        # (r, k, d) -> (k, r*d) -> (k, g, j) -> (g, k, j) -> (120, 29400)
        u = u.transpose(1, 0, 2).reshape(NBLK, NG, PW).transpose(1, 0, 2)
        u = np.ascontiguousarray(u).reshape(P, PW).astype(BF16_NP)
        m = {"a": _A_BF16}
        t0 = 0
        for g, t1 in enumerate(_LOAD_HI):
            m[f"x{g}"] = np.ascontiguousarray(u[:, t0 * TN : t1 * TN])
            t0 = t1
        maps.append(m)
    return maps


def _unshard(results: list[dict[str, np.ndarray]]) -> np.ndarray:
    outs = []
    for r in results:
        yc = np.concatenate(
            [np.asarray(r[f"y{s}"]) for s in range(len(STORE_TILES))], axis=1
        ).astype(np.float32)
        yc = yc.reshape(NG, NBLK, PW)
        yc = yc.transpose(1, 0, 2).reshape(NBLK, ROWS, D).transpose(1, 0, 2)
        outs.append(np.ascontiguousarray(yc).reshape(ROWS, T))
    return np.concatenate(outs, axis=0)


def kernel(x: np.ndarray) -> np.ndarray:
    nc = _get_nc()
    res = run_bass_kernel_spmd(nc, _shard(x), core_ids=list(range(NCORES)))
    return _unshard(res.results)


def kernel_profiled(x: np.ndarray):
    """Like kernel() but with NTFF tracing; returns (out, BassKernelResults)."""
    nc = _get_nc()
    res = run_bass_kernel_spmd(
        nc, _shard(x), core_ids=list(range(NCORES)), trace=True
    )
    return _unshard(res.results), res


# revision 35
# speedup vs baseline: 1.1323x; 1.1323x over previous
"""Trainium2 Bass kernel for nn_BatchDelayProcessor.

Computes, per batch row (B=64, T=441000, D=22050 delay, 20 blocks):
    delayed[t] = 0                          , t < D
    delayed[t] = x[t-D] + 0.3*delayed[t-D]  , t >= D
    out[t]     = 0.5*x[t] + 0.5*delayed[t]

With u_k = 0.5*x_k (folded into the host-side bf16 conversion -- an exact
exponent shift), the block recurrence unrolls to a dense lower-triangular
combination:
    out_m = u_m + sum_{j<m} 0.3^(m-1-j) * u_j  =  sum_j A[m,j] u_j
so the whole kernel is ONE 20x20 matrix applied per sample position --
perfect for the otherwise-idle PE array, with NO serial chain anywhere.
Six independent position-groups are folded into a 120x120 block-diagonal
stationary, so each matmul tile computes 6 groups x 20 blocks at once.

Pipeline (per core): loads -> PE (60 tiles of 490 cols) -> PSUM drain
(split DVE even tiles / ACT odd tiles, f32->bf16 downcast) -> stores.

Schedule notes (evidence from perfetto traces of 8 prior variants):
  - bf16 I/O halves HBM traffic to 7.06 MB each way (tol 2e-2, measured
    err 3.8e-3).
  - ALL DMA on the single sync HWDGE ring, loads (consumption order,
    telescoping sizes) ahead of stores (drain-gated): HWDGE descriptors
    execute at full ~26.5 GB/s/engine vs ~15 for SWDGE ones, one ring's
    strict FIFO avoids the cross-queue round-robin that starved head
    groups in two-queue variants, and loads-then-stores serializes HBM
    into pure-read then pure-write phases.
  - Each load/store group is its OWN dense DRAM parameter (120 x cols
    contiguous): with a strided (120, 29400) layout, per-descriptor
    reads jumped 58.8 KB apart and HBM read rate dropped to ~15
    GB/s/engine; dense per-DMA regions restore line rate.

Sharding: data-parallel over batch -- 8 rows per NeuronCore, no comms.
Layout: partition p = g*20 + k holds block k of position-group g; its
29400 columns are sample positions (flattened (row, d), split into 6
contiguous groups of 29400).
"""

from contextlib import ExitStack

import ml_dtypes
import numpy as np

import concourse.bass as bass
import concourse.mybir as mybir
from concourse.bass_utils import run_bass_kernel_spmd

B, T = 64, 441000
D, NBLK = 22050, 20
NCORES = 8
ROWS = B // NCORES          # 8 rows per core
NG = 6                      # position-groups folded into the stationary
P = NG * NBLK               # 120 partitions
PW = ROWS * D // NG         # 29400 positions per group-lane

TN = 490                    # moving-tile columns (1960 B psum = 1 bank)
NT = PW // TN               # 60 tiles
NPB = 8                     # psum bank ring

# telescoping load groups (units: tiles); small at BOTH ends: fast PE
# start, and only ~2us of PE work left after the last load byte lands
# (a 12-tile last group left ~4.4us of tail compute)
LOAD_TILES = [3, 3, 6, 12, 12, 12, 6, 3, 3]
# store groups (tiles); boundaries must be even (drain-parity gating);
# tiny last group shrinks the final write-receipt tail
STORE_TILES = [12, 12, 12, 12, 6, 4, 2]
FEEDBACK, MIX = 0.3, 0.5

BF16 = mybir.dt.bfloat16
F32 = mybir.dt.float32
BF16_NP = ml_dtypes.bfloat16


def _amat() -> np.ndarray:
    """Block-diag stationary lhsT[(g,k),(g',m)] = A[m,k] * (g==g')."""
    A = np.zeros((NBLK, NBLK), dtype=np.float64)
    for m in range(NBLK):
        A[m, m] = 1.0
        for j in range(m):
            A[m, j] = FEEDBACK ** (m - 1 - j)
    lhsT = np.zeros((P, P), dtype=np.float64)
    for g in range(NG):
        lhsT[g * NBLK : (g + 1) * NBLK, g * NBLK : (g + 1) * NBLK] = A.T
    return lhsT.astype(BF16_NP)


def build_nc() -> bass.Bass:
    assert sum(LOAD_TILES) == NT and sum(STORE_TILES) == NT
    load_hi = np.cumsum(LOAD_TILES).tolist()      # exclusive upper tile
    store_lo = [0] + np.cumsum(STORE_TILES).tolist()[:-1]

    nc = bass.Bass(trn_type="TRN2")
    a = nc.declare_dram_parameter("a", [P, P], BF16, isOutput=False)
    # one dense DRAM tensor per load group / store group
    xs = [
        nc.declare_dram_parameter(f"x{g}", [P, nt * TN], BF16, isOutput=False)
        for g, nt in enumerate(LOAD_TILES)
    ]
    ys = [
        nc.declare_dram_parameter(f"y{s}", [P, nt * TN], BF16, isOutput=True)
        for s, nt in enumerate(STORE_TILES)
    ]

    with ExitStack() as ctx:
        block = ctx.enter_context(nc.Block())
        abuf = ctx.enter_context(nc.sbuf_tensor("abuf", [P, P], BF16))
        xbuf = ctx.enter_context(nc.sbuf_tensor("xbuf", [P, PW], BF16))
        obuf = ctx.enter_context(nc.sbuf_tensor("obuf", [P, PW], BF16))
        pb = [
            ctx.enter_context(nc.psum_tensor(f"pb{j}", [P, TN], F32))
            for j in range(NPB)
        ]
        s_a = ctx.enter_context(nc.semaphore("s_a"))
        s_in = [
            ctx.enter_context(nc.semaphore(f"s_in{g}"))
            for g in range(len(LOAD_TILES))
        ]
        s_pe = ctx.enter_context(nc.semaphore("s_pe"))
        s_dve = ctx.enter_context(nc.semaphore("s_dve"))
        s_act = ctx.enter_context(nc.semaphore("s_act"))
        s_st = ctx.enter_context(nc.semaphore("s_st"))

        def tcols(buf, t0, t1):
            return buf[:, t0 * TN : t1 * TN]

        # Loads are split BY PARTITION HALVES across the two HWDGE
        # rings: both rings carry identical byte schedules in the same
        # group order, and the PE waits for both halves (sem >= 32), so
        # cross-queue arbitration skew cannot starve the next-needed
        # group (which is what sank every BY-GROUP two-queue split).
        # This probes whether the ~200 GB/s read rate is a per-queue
        # limit; stores queue on the sync ring behind its half-loads.
        PH = P // 2

        @block.sync
        def _(sync):
            sync.dma_start(out=abuf[:, :], in_=a[:, :]).then_inc(s_a, 16)
            t0 = 0
            for g, t1 in enumerate(load_hi):
                sync.dma_start(
                    out=xbuf[0:PH, t0 * TN : t1 * TN],
                    in_=xs[g][0:PH, :],
                ).then_inc(s_in[g], 16)
                t0 = t1
            for s, st0 in enumerate(store_lo):
                st1 = st0 + STORE_TILES[s]
                sync.wait_ge(s_dve, st1 // 2)
                sync.wait_ge(s_act, st1 // 2)
                sync.dma_start(
                    out=ys[s][0:PH, :],
                    in_=obuf[0:PH, st0 * TN : st1 * TN],
                ).then_inc(s_st, 16)

        # PE: one self-loading matmul per 490-column tile against the
        # block-diagonal stationary; psum banks cycle mod 8.
        @block.tensor
        def _(tensor):
            tensor.wait_ge(s_a, 16)
            g = -1
            for t in range(NT):
                gt = next(i for i, hi in enumerate(load_hi) if t < hi)
                if gt > g:
                    tensor.wait_ge(s_in[gt], 32)  # both partition halves
                    g = gt
                if t >= NPB:
                    td = t - NPB  # bank WAR: tile td's drain read the bank
                    if td % 2 == 0:
                        tensor.wait_ge(s_dve, td // 2 + 1)
                    else:
                        tensor.wait_ge(s_act, td // 2 + 1)
                nc.tensor.matmul(
                    out=pb[t % NPB][:, :],
                    lhsT=abuf[:, :],
                    rhs=tcols(xbuf, t, t + 1),
                    start=True,
                    stop=True,
                ).then_inc(s_pe, 1)

        # PSUM drain, split across DVE (even tiles) and ACT (odd tiles);
        # both do the f32 -> bf16 downcast into obuf.
        @block.vector
        def _(vector):
            for t in range(0, NT, 2):
                vector.wait_ge(s_pe, t + 1)
                nc.vector.tensor_copy(
                    tcols(obuf, t, t + 1), pb[t % NPB][:, :]
                ).then_inc(s_dve, 1)

        @block.scalar
        def _(scalar):
            # upper partition halves of every load group, same order as
            # the sync ring's lower halves; all emitted up-front before
            # the drain loop needs this sequencer
            t0 = 0
            for g, t1 in enumerate(load_hi):
                scalar.dma_start(
                    out=xbuf[PH:P, t0 * TN : t1 * TN],
                    in_=xs[g][PH:P, :],
                ).then_inc(s_in[g], 16)
                t0 = t1
            # upper-half stores ride this ring too (balances total ring
            # bytes ~7.1 MB each); a group covering tiles [st0, st1) is
            # emitted after this sequencer's own drain of tile st1-1,
            # with explicit drain waits before the DMA reads obuf
            emit_after = {
                st0 + STORE_TILES[s] - 1: s for s, st0 in enumerate(store_lo)
            }
            for t in range(1, NT, 2):
                scalar.wait_ge(s_pe, t + 1)
                nc.scalar.copy(
                    tcols(obuf, t, t + 1), pb[t % NPB][:, :]
                ).then_inc(s_act, 1)
                if t in emit_after:
                    s = emit_after[t]
                    st0 = store_lo[s]
                    st1 = st0 + STORE_TILES[s]
                    scalar.wait_ge(s_dve, st1 // 2)
                    scalar.wait_ge(s_act, st1 // 2)
                    scalar.dma_start(
                        out=ys[s][PH:P, :],
                        in_=obuf[PH:P, st0 * TN : st1 * TN],
                    ).then_inc(s_st, 16)

    return nc


_NC_CACHE = None


def _get_nc() -> bass.Bass:
    global _NC_CACHE
    if _NC_CACHE is None:
        _NC_CACHE = build_nc()
    return _NC_CACHE


_A_BF16 = _amat()
_LOAD_HI = np.cumsum(LOAD_TILES).tolist()
_STORE_HI = np.cumsum(STORE_TILES).tolist()


def _shard(x: np.ndarray) -> list[dict[str, np.ndarray]]:
    x = np.asarray(x, dtype=np.float32)
    assert x.shape == (B, T), x.shape
    maps = []
    for i in range(NCORES):
        u = (x[i * ROWS : (i + 1) * ROWS] * np.float32(MIX)).reshape(
            ROWS, NBLK, D
        )
        # (r, k, d) -> (k, r*d) -> (k, g, j) -> (g, k, j) -> (120, 29400)
        u = u.transpose(1, 0, 2).reshape(NBLK, NG, PW).transpose(1, 0, 2)
        u = np.ascontiguousarray(u).reshape(P, PW).astype(BF16_NP)
        m = {"a": _A_BF16}
        t0 = 0
        for g, t1 in enumerate(_LOAD_HI):
            m[f"x{g}"] = np.ascontiguousarray(u[:, t0 * TN : t1 * TN])
            t0 = t1
        maps.append(m)
    return maps


def _unshard(results: list[dict[str, np.ndarray]]) -> np.ndarray:
    outs = []
    for r in results:
        yc = np.concatenate(
            [np.asarray(r[f"y{s}"]) for s in range(len(STORE_TILES))], axis=1
        ).astype(np.float32)
        yc = yc.reshape(NG, NBLK, PW)
        yc = yc.transpose(1, 0, 2).reshape(NBLK, ROWS, D).transpose(1, 0, 2)
        outs.append(np.ascontiguousarray(yc).reshape(ROWS, T))
    return np.concatenate(outs, axis=0)


def kernel(x: np.ndarray) -> np.ndarray:
    nc = _get_nc()
    res = run_bass_kernel_spmd(nc, _shard(x), core_ids=list(range(NCORES)))
    return _unshard(res.results)


def kernel_profiled(x: np.ndarray):
    """Like kernel() but with NTFF tracing; returns (out, BassKernelResults)."""
    nc = _get_nc()
    res = run_bass_kernel_spmd(
        nc, _shard(x), core_ids=list(range(NCORES)), trace=True
    )
    return _unshard(res.results), res
